# revision 47
# baseline (speedup 1.0000x reference)
"""Trainium2 Bass kernel for nn_CustomCNN (LeNet-style CNN, batch 8192).

Strategy (pure data parallel over 8 cores, 1024 images each, 8 blocks of 128):
- The whole pipeline runs in fp16 (10-bit mantissa) with fp32 PSUM
  accumulation: measured end-to-end rel err ~6.6e-3 vs the 2e-2 budget.
  fp16 halves the x DMA and SBUF traffic, gives DVE 2x_1p throughput on the
  pool adds, and runs 1 cyc/row on the PE at any moving size.
- x is host-relaid per core as [block, 128 feat-partitions, 3072] fp16 with
  features row-major channel-interleaved (f = r*96 + w*3 + c), so each
  block's load is one fully-contiguous [128 x 6KB] DMA burst and the first
  chunks of block 0 can land early (split head DMA).
- conv1 as batch-in-M matmuls: each single-output-row region (N=168)
  accumulates over 4-5 of the 24 128-feature chunks (4.5 avg; alignment
  repeats with period 4, so 20 prebuilt sparse weight mats); 126 matmuls x
  168 cyc per block vs 84 x 336 for 4-row/channel-split chunking (-25%).
  Rows are packed two-per-1-bank-PSUM-tile using deferred zero-region
  semantics (start only on the pair's first matmul marks the whole 2KB
  region pending-zero; stop only on its last), chunk-major emission
  amortizes LDWEIGHTS, and each pair finishes with one 336-elem tanh.
- x block loads ride the sync (SP) DMA ring and const loads the Pool ring:
  the Activation engine (the #2 bottleneck at ~70% busy) only runs tanh.
- The bugged avgpool (channel-mean + 2x2) collapses conv2/conv3 to
  single-channel kernels; pool scale factors are folded into next-layer
  weights. pool1 is split on the conv2 stationary windows (s1 rows 0..8 /
  9..13) so conv2 row-half 0 runs while conv1 finishes the same block.
- Schedule per block: chunks 0..16 | conv2-ch1(prev) | chunks 17..19 |
  pool1-A | pool2-finish+tail(prev) | conv2-ch0 + pool2-ch0-tree |
  chunks 20..23 | pool1-B -- every cross-engine consumer has a slab of
  conv1 matmuls in front of it (sim: 87% PE occupancy); pool2's channel
  tree is split per conv2 row-half so half 0 runs mid-block, and group
  tails run 3+1 so only a thin 128-wide conv3..fc2 chain ends the kernel.
- pool2 runs on the (otherwise idle) GPSIMD engine; tails are feature-major
  over groups of 4 blocks: conv3/fc1 as single N=512 matmuls, fc1 bias via
  the activation bias port, fc2 uses the activation tile as the stationary
  operand to come back image-major; each group's outputs DMA out eagerly.
"""

import sys

import numpy as np

if "/opt/trn_rl_repo" not in sys.path:
    sys.path.insert(0, "/opt/trn_rl_repo")

NCORES = 8
BPC = 1024          # images per core
NBLK = 8            # blocks of 128 images per core
P = 128

_CACHE = {}


def _build_weight_mats(k1, k2, k3, W1, b1, W2, b2):
    """Host-side construction of the dense matmul operand matrices."""
    f32 = np.float32
    f16 = np.float16
    k1 = np.asarray(k1, f32)
    k2e = (np.asarray(k2, f32).sum(1) / 24.0).astype(f32)   # [16,5,5] pool1 scale folded
    k3e = (np.asarray(k3, f32).sum(1) / 64.0).astype(f32)   # [120,5,5] pool2 scale folded

    # conv1 (fp16 operands: the all-fp16 pipeline measures ~7e-3 rel err
    # vs the 2e-2 budget, and fp16 halves the x DMA): x features are
    # row-major channel-interleaved (f = r*96 + w*3 + c), cut into 24
    # chunks of 128. Each single-output-row group r needs input rows
    # r..r+4 (480 feats); its chunk window depends only on the phase
    # q = r%4: start chunk 3*(r//4) + [0,0,1,2][q], window offset
    # [0,96,64,32][q], spanning 4 chunks for q in {0,3} and 5 for
    # q in {1,2} -- avg 4.5 passes/row vs 5 for 2-row groups (fp16 runs
    # 1 cyc/row at any N). W1m[q, j][p, col=(oc*28 + ow)].
    W1m = np.zeros((4, 5, 128, 168), f16)
    offq = [0, 96, 64, 32]
    for q in range(4):
        for j in range(5):
            for p in range(128):
                wo = 128 * j + p - offq[q]     # offset into the 5-row window
                if not (0 <= wo < 480):
                    continue
                kh, rem = divmod(wo, 96)       # input row rel. to r, col, ch
                w, c = divmod(rem, 3)
                for oc in range(6):
                    for ow in range(28):
                        kw = w - ow
                        if 0 <= kw < 5:
                            W1m[q, j, p, oc * 28 + ow] = k1[oc, c, kh, kw]

    # conv2 (collapsed): input s1 [14,14]; chunk = 9 rows x 14 cols = 126 feats.
    # W2m[row=(rt*14+w), col=(oc*50 + ohl*10 + ow)] ; rows 126/127 zero-padded.
    W2m = np.zeros((128, 800), f16)
    for oc in range(16):
        for ohl in range(5):
            for ow in range(10):
                col = oc * 50 + ohl * 10 + ow
                for kh in range(5):
                    rt = ohl + kh          # 0..8
                    for kw in range(5):
                        W2m[rt * 14 + ow + kw, col] = k2e[oc, kh, kw]

    # conv3 (collapsed to matmul): s2 [25] -> 120
    K3m = np.zeros((25, 120), f16)
    for o in range(120):
        K3m[:, o] = k3e[o].reshape(25)

    return {
        "w1m": W1m,
        "w2m": W2m,
        "k3m": K3m,
        "fc1": np.asarray(W1, f16),                                  # [120, 84]
        "b1c": np.asarray(b1, f32).reshape(84, 1),                   # [84, 1]
        "fc2": np.asarray(W2, f16),                                  # [84, 10]
        "b2r": np.tile(np.asarray(b2, f32).reshape(1, 10), (128, 1)),  # [128, 10]
        "ident": np.eye(128, dtype=f16),
    }


def _relayout_x(x_core):
    """[n*128, 3, 32, 32] -> [n blk, 128 part, 24*128]: feature order is
    row-major channel-interleaved (f = r*96 + w*3 + c), cut into 24 chunks
    of 128 features; chunk k, partition p holds feature 128k+p, image i."""
    x_core = np.asarray(x_core, np.float16)
    nblk = x_core.shape[0] // 128
    xr = x_core.reshape(nblk, 128, 3, 32, 32)
    # [b, i, c, r, w] -> [b, r, w, c, i] -> [b, 24k, 128p, i] -> [b, p, k, i]
    xt = xr.transpose(0, 3, 4, 2, 1).reshape(nblk, 24, 128, 128)
    xt = xt.transpose(0, 2, 1, 3)
    return np.ascontiguousarray(xt.reshape(nblk, 128, 3072))


def _build_bass(n_blocks=NBLK, n_reps=1):
    import concourse.bass as bass
    import concourse.bacc as bacc
    import concourse.mybir as mybir
    import concourse.tile as tile

    f32 = mybir.dt.float32
    f32r = mybir.dt.float32r
    f16 = mybir.dt.float16
    TANH = mybir.ActivationFunctionType.Tanh
    MS = bass.MemorySpace

    nc = bacc.Bacc("TRN2", target_bir_lowering=False, debug=False,
                   num_devices=NCORES)

    x_d = nc.dram_tensor("x", [n_blocks, 128, 3072], f16, kind="ExternalInput")
    w1_d = nc.dram_tensor("w1m", [4, 5, 128, 168], f16, kind="ExternalInput")
    w2_d = nc.dram_tensor("w2m", [128, 800], f16, kind="ExternalInput")
    k3_d = nc.dram_tensor("k3m", [25, 120], f16, kind="ExternalInput")
    fc1_d = nc.dram_tensor("fc1", [120, 84], f16, kind="ExternalInput")
    b1_d = nc.dram_tensor("b1c", [84, 1], f32, kind="ExternalInput")
    fc2_d = nc.dram_tensor("fc2", [84, 10], f16, kind="ExternalInput")
    b2_d = nc.dram_tensor("b2r", [128, 10], f32, kind="ExternalInput")
    id_d = nc.dram_tensor("ident", [128, 128], f16, kind="ExternalInput")
    out_d = nc.dram_tensor("out", [n_blocks * P, 10], f32, kind="ExternalOutput")

    nvb = n_blocks * n_reps

    with tile.TileContext(nc) as tc:
        with (
            tc.tile_pool(name="consts", bufs=1) as consts,
            tc.tile_pool(name="chk", bufs=2) as chk,
            tc.tile_pool(name="t1p", bufs=2) as t1p,
            tc.tile_pool(name="tmp1", bufs=2) as tmp1,
            tc.tile_pool(name="s1p", bufs=2) as s1p,
            tc.tile_pool(name="s1Tp", bufs=2) as s1Tp,
            tc.tile_pool(name="t2p", bufs=2) as t2p,
            tc.tile_pool(name="tmp2", bufs=2) as tmp2,
            tc.tile_pool(name="s2ap", bufs=2) as s2ap,
            tc.tile_pool(name="s2Tp", bufs=2) as s2Tp,
            tc.tile_pool(name="t3p", bufs=2) as t3p,
            tc.tile_pool(name="t4p", bufs=2) as t4p,
            tc.tile_pool(name="outp", bufs=1) as outp,
            tc.tile_pool(name="ps1", bufs=4, space=MS.PSUM) as ps1p,
            tc.tile_pool(name="ps2", bufs=1, space=MS.PSUM) as ps2p,
            tc.tile_pool(name="ptT", bufs=1, space=MS.PSUM) as ptTp,
            tc.tile_pool(name="pss", bufs=1, space=MS.PSUM) as pss,
        ):
            # ---- constants into SBUF (once, on the idle Pool DMA ring) ----
            w1sb = consts.tile([128, 20 * 168], f16, tag="w1sb")
            for q in range(4):
                for j in range(5):
                    k = q * 5 + j
                    nc.gpsimd.dma_start(w1sb[:, k * 168:(k + 1) * 168], w1_d[q, j])
            w2sb = consts.tile([128, 800], f16, tag="w2sb")
            nc.gpsimd.dma_start(w2sb[:], w2_d[:])
            k3sb = consts.tile([128, 120], f16, tag="k3sb")
            nc.gpsimd.dma_start(k3sb[0:25, :], k3_d[:])
            fc1sb = consts.tile([128, 84], f16, tag="fc1sb")
            nc.gpsimd.dma_start(fc1sb[0:120, :], fc1_d[:])
            b1sb = consts.tile([128, 1], f32, tag="b1sb")
            nc.gpsimd.dma_start(b1sb[0:84, :], b1_d[:])
            fc2sb = consts.tile([128, 10], f16, tag="fc2sb")
            nc.gpsimd.dma_start(fc2sb[0:84, :], fc2_d[:])
            b2sb = consts.tile([128, 10], f32, tag="b2sb")
            nc.gpsimd.dma_start(b2sb[:], b2_d[:])
            ident = consts.tile([128, 128], f16, tag="ident")
            nc.gpsimd.dma_start(ident[:], id_d[:])
            out_sb = outp.tile([128, n_blocks * 10], f32, tag="outsb")

            def w1t(q, j):
                k = q * 5 + j
                return w1sb[:, k * 168:(k + 1) * 168]

            K0Q = [0, 0, 1, 2]
            PQ = [4, 5, 5, 4]

            s1h = {}            # vb -> pool1/t1 tiles
            c2s = {}            # vb -> conv2 in-flight tiles
            grp = {}            # g -> {"ptT":, "s2T":, "n":}

            def pool1_part(st, part):
                """pool1 for s1 rows 0..8 (part 0) / 9..13 (part 1); fp16 2x.

                Split on the conv2 stationary windows (rows 0..8 / 5..13) so
                conv2 row-half 0 can run while conv1 finishes the block."""
                r0, r1 = (0, 18) if part == 0 else (18, 28)
                i0, i1 = r0 // 2, r1 // 2
                t1h, u1, u, v, s1 = (st[k] for k in ("t1h", "u1", "u", "v", "s1"))
                u1h = u1[:].rearrange("p (ocl oh ow) -> p ocl oh ow", oh=28, ow=28)
                nc.vector.tensor_add(u1h[:, :, r0:r1, :], t1h[:, 0:3, r0:r1, :],
                                     t1h[:, 3:6, r0:r1, :])
                uh = u[:].rearrange("p (oh ow) -> p oh ow", ow=28)
                nc.vector.tensor_add(uh[:, r0:r1, :], u1h[:, 0, r0:r1, :],
                                     u1h[:, 1, r0:r1, :])
                nc.vector.tensor_add(uh[:, r0:r1, :], uh[:, r0:r1, :],
                                     u1h[:, 2, r0:r1, :])
                ur = u[:].rearrange("p (i t w) -> p t i w", t=2, w=28)   # i=14
                vr = v[:].rearrange("p (i w) -> p i w", w=28)
                nc.vector.tensor_add(vr[:, i0:i1, :], ur[:, 0, i0:i1, :],
                                     ur[:, 1, i0:i1, :])
                v2 = v[:].rearrange("p (i j t) -> p t i j", t=2, j=14)
                s1r = s1[:].rearrange("p (i j) -> p i j", j=14)
                nc.vector.tensor_add(s1r[:, i0:i1, :], v2[:, 0, i0:i1, :],
                                     v2[:, 1, i0:i1, :])

            def conv2_half(vb, ch):
                """Row-half ch of conv2: transpose s1 rows 5ch..5ch+8, then 2
                matmuls (oc-halves) + 1 tanh of 800 into t2 rows 5ch..5ch+4.

                ps2p has bufs=1 (2 PSUM banks): the ping-pong between halves
                is hidden by the conv1 slab emitted between them."""
                st = c2s[vb]
                ptc = pss.tile([128, 512], f16, tag="pss")
                nc.tensor.transpose(ptc[0:126, 0:128],
                                    st["s1"][:, ch * 70:ch * 70 + 126], ident[:])
                s1T = s1Tp.tile([128, 128], f16, tag="s1T")
                nc.vector.tensor_copy(s1T[0:126, :], ptc[0:126, 0:128])
                ps2 = ps2p.tile([128, 1024], f32, tag="ps2", name="ps2")
                for h2 in range(2):
                    nc.tensor.matmul(ps2[:, h2 * 512:h2 * 512 + 400],
                                     s1T[0:126, :],
                                     w2sb[0:126, h2 * 400:(h2 + 1) * 400])
                t2r = st["t2"][:].rearrange("p (oc oh ow) -> p oc oh ow",
                                            oh=10, ow=10)
                dst = t2r[:, :, 5 * ch:5 * ch + 5, :]
                srcap = ps2[:].rearrange("p (h x) -> p h x", h=2)[:, :, 0:400]
                srcap = srcap.rearrange("p h (ocl ohl ow) -> p h ocl ohl ow",
                                        ohl=5, ow=10)
                nc.scalar.activation(dst, srcap, TANH)


            def pool2_half(vb, ch):
                """Channel tree of pool2 over t2 rows 5ch..5ch+4 (GPSIMD) +
                the within-half output-row-pair sums; runs behind conv2-ch."""
                st = c2s[vb]
                t2r = st["t2"][:].rearrange("p (oc oh ow) -> p oc oh ow",
                                            oh=10, ow=10)
                r0, r1 = 5 * ch, 5 * ch + 5
                a2r = st["a2"][:].rearrange("p (oc oh ow) -> p oc oh ow",
                                            oh=10, ow=10)          # [8,10,10]
                nc.gpsimd.tensor_add(a2r[:, :, r0:r1, :],
                                     t2r[:, 0:8, r0:r1, :], t2r[:, 8:16, r0:r1, :])
                b2r_ = st["b2t"][:].rearrange("p (oc oh ow) -> p oc oh ow",
                                              oh=10, ow=10)        # [4,10,10]
                nc.gpsimd.tensor_add(b2r_[:, :, r0:r1, :],
                                     a2r[:, 0:4, r0:r1, :], a2r[:, 4:8, r0:r1, :])
                c2r = st["c2t"][:].rearrange("p (oc oh ow) -> p oc oh ow",
                                             oh=10, ow=10)         # [2,10,10]
                nc.gpsimd.tensor_add(c2r[:, :, r0:r1, :],
                                     b2r_[:, 0:2, r0:r1, :], b2r_[:, 2:4, r0:r1, :])
                d2r = st["d2t"][:].rearrange("p (oh ow) -> p oh ow", ow=10)
                nc.gpsimd.tensor_add(d2r[:, r0:r1, :],
                                     c2r[:, 0, r0:r1, :], c2r[:, 1, r0:r1, :])
                # within-half output-row pairs -> e2 rows (0,1) / (3,4)
                e2r = st["e2"][:].rearrange("p (i w) -> p i w", w=10)  # [5,10]
                if ch == 0:
                    nc.gpsimd.tensor_add(e2r[:, 0:2, :],
                                         d2r[:, 0:4:2, :], d2r[:, 1:4:2, :])
                else:
                    nc.gpsimd.tensor_add(e2r[:, 3:5, :],
                                         d2r[:, 6:10:2, :], d2r[:, 7:10:2, :])

            def emit_finishB(vb):
                """pool2 finish + s2 member transpose; members 0-2 tail when
                the 3rd lands (mid-pipeline), member 3 in a thin end chain."""
                pool2_half(vb, 1)
                st = c2s.pop(vb)
                g = vb // 4
                if g not in grp:
                    grp[g] = {"ptT": ptTp.tile([128, 512], f16, tag="ptT",
                                               name="ptT"),
                              "s2T": s2Tp.tile([128, 512], f16, tag="s2T",
                                               name="s2T"),
                              "n": 0}
                gs = grp[g]
                m = gs["n"]
                gs["n"] += 1
                if m == 0:
                    gs["b0"] = vb % n_blocks

                # pool2: half trees already ran behind the conv2 halves;
                # finish the cross-half row pair and the 2x2 col pairs.
                d2r = st["d2t"][:].rearrange("p (oh ow) -> p oh ow", ow=10)
                e2 = st["e2"]
                e2r = e2[:].rearrange("p (i w) -> p i w", w=10)
                nc.gpsimd.tensor_add(e2r[:, 2:3, :], d2r[:, 4:5, :],
                                     d2r[:, 5:6, :])
                e2v = e2[:].rearrange("p (i j t) -> p t i j", t=2, j=5)
                s2m = s2ap.tile([128, 32], f16, tag="s2m")
                s2r = s2m[:, 0:25].rearrange("p (i j) -> p i j", j=5)
                nc.gpsimd.tensor_add(s2r, e2v[:, 0], e2v[:, 1])

                # member transpose + copy into the group's s2T immediately
                nc.tensor.transpose(gs["ptT"][0:25, 128 * m:128 * (m + 1)],
                                    s2m[:, 0:25], ident[:])
                nc.vector.tensor_copy(gs["s2T"][0:25, 128 * m:128 * (m + 1)],
                                      gs["ptT"][0:25, 128 * m:128 * (m + 1)])

                if gs["n"] == 3:
                    emit_tailB(g, 0, 3)     # members 0-2 ride mid-pipeline
                elif gs["n"] == 4:
                    emit_tailB(g, 3, 4)     # thin 128-wide chain at the end

            def emit_tailB(g, lo, hi):
                """conv3..fc2 + out DMA for group members [lo, hi)."""
                gs = grp[g]
                gs["tailed"] = hi
                s2T = gs["s2T"]
                c0, c1 = 128 * lo, 128 * hi
                W = c1 - c0

                # ---- conv3 (25->120) + tanh3, feature-major ----
                ps3 = pss.tile([128, 512], f32, tag="pss")
                nc.tensor.matmul(ps3[0:120, 0:W], k3sb[0:25, 0:120],
                                 s2T[0:25, c0:c1])
                t3a = t3p.tile([128, 512], f16, tag="t3a")
                nc.scalar.activation(t3a[0:120, 0:W], ps3[0:120, 0:W], TANH)

                # ---- fc1 + tanh4 (bias via per-partition activation bias) ----
                ps4 = pss.tile([128, 512], f32, tag="pss")
                nc.tensor.matmul(ps4[0:84, 0:W], fc1sb[0:120, 0:84],
                                 t3a[0:120, 0:W])
                t4a = t4p.tile([128, 512], f16, tag="t4a")
                nc.scalar.activation(t4a[0:84, 0:W], ps4[0:84, 0:W], TANH,
                                     bias=b1sb[0:84, 0:1])

                # ---- fc2: activation tile as stationary -> image-major out ----
                ps5 = pss.tile([128, 512], f32, tag="pss")
                for m in range(lo, hi):
                    nc.tensor.matmul(ps5[:, 10 * m:10 * (m + 1)],
                                     t4a[0:84, 128 * (m - lo):128 * (m - lo + 1)],
                                     fc2sb[0:84, :])
                b0 = gs["b0"]
                for m in range(lo, hi):
                    b = b0 + m
                    nc.vector.tensor_add(out_sb[:, b * 10:(b + 1) * 10],
                                         ps5[:, 10 * m:10 * (m + 1)], b2sb[:])
                # ship these blocks' outputs right away
                od = out_d[:].rearrange("(blk p) f -> p blk f", p=P)
                ob = out_sb[:].rearrange("p (blk f) -> p blk f", f=10)
                nc.sync.dma_start(od[:, b0 + lo:b0 + hi, :],
                                  ob[:, b0 + lo:b0 + hi, :])

            def conv1_chunks(vb, st, k_lo, k_hi):
                """1-row accumulation regions (N=168), paired two-per-1-bank
                tile (rows 2m, 2m+1 at cols 0:168 / 168:336). PSUM zeroing is
                deferred per zero-region: ONLY the pair's first matmul sets
                start (marking the whole 2KB region pending-zero, so each
                row's first write overwrites) and only the pair's last sets
                stop. At most 4 pair tiles are open; each closes with one
                336-elem tanh at chunk 3*(m//2) + 4 + (m%2)."""
                chunks, psg, t1h = st["chunks"], st["psg"], st["t1h"]
                for k in range(k_lo, k_hi):
                    ck = chunks[:, k * 128:(k + 1) * 128]
                    for r in range(28):
                        t, q = divmod(r, 4)
                        j = k - (3 * t + K0Q[q])
                        if not (0 <= j < PQ[q]):
                            continue
                        m = r // 2
                        first = (r % 2 == 0) and j == 0
                        last = (r % 2 == 1) and j == PQ[q] - 1
                        if first:
                            psg[m] = ps1p.tile([128, 336], f32, tag="ps1",
                                               name="ps1")
                        col0 = 168 * (r % 2)
                        nc.tensor.matmul(psg[m][:, col0:col0 + 168],
                                         ck, w1t(q, j),
                                         start=first, stop=last)
                    for m in range(14):
                        if 3 * (m // 2) + 4 + (m % 2) == k:
                            ps = psg.pop(m)
                            dst = t1h[:, :, 2 * m:2 * m + 2, :]
                            srcap = ps[:].rearrange(
                                "p (rl oc ow) -> p oc rl ow", rl=2, ow=28)
                            nc.scalar.activation(dst, srcap, TANH)

            def conv1_open(vb):
                b = vb % n_blocks
                chunks = chk.tile([128, 3072], f16, tag="chunks")
                # x loads ride the sync (SP) DMA ring; the ACT engine only
                # runs tanh. Block 0 is split so early chunks arrive first.
                if vb == 0:
                    nc.sync.dma_start(chunks[:, 0:768], x_d[b, :, 0:768])
                    nc.sync.dma_start(chunks[:, 768:1792], x_d[b, :, 768:1792])
                    nc.sync.dma_start(chunks[:, 1792:3072], x_d[b, :, 1792:3072])
                else:
                    nc.sync.dma_start(chunks[:], x_d[b])
                t1 = t1p.tile([128, 4704], f16, tag="t1")
                st = {
                    "chunks": chunks,
                    "psg": {},
                    "t1h": t1[:].rearrange("p (oc oh ow) -> p oc oh ow",
                                           oh=28, ow=28),
                    "u1": tmp1.tile([128, 2352], f16, tag="u1", name="u1"),
                    "u": tmp1.tile([128, 784], f16, tag="u", name="u"),
                    "v": tmp1.tile([128, 392], f16, tag="v", name="v"),
                    "s1": s1p.tile([128, 196], f16, tag="s1", name="s1"),
                    "t2": t2p.tile([128, 1600], f16, tag="t2", name="t2"),
                    "a2": tmp2.tile([128, 800], f16, tag="a2", name="a2"),
                    "b2t": tmp2.tile([128, 400], f16, tag="b2t", name="b2t"),
                    "c2t": tmp2.tile([128, 200], f16, tag="c2t", name="c2t"),
                    "d2t": tmp2.tile([128, 100], f16, tag="d2t", name="d2t"),
                    "e2": tmp2.tile([128, 50], f16, tag="e2", name="e2"),
                }
                s1h[vb] = st
                c2s[vb] = st
                return st

            # ---- software-pipelined emission ----
            # Per iteration (block vb):
            #   chunks 0..16 | conv2-ch1(vb-1) | chunks 17..19 + poolA |
            #   pool2(vb-1) + group tail | conv2-ch0(vb) | chunks 20..23 |
            #   poolB(vb)
            # so every cross-engine consumer has a slab of conv1 matmuls in
            # front of it on the PE queue.
            seq = list(range(nvb))
            for i, vb in enumerate(seq):
                st = conv1_open(vb)
                conv1_chunks(vb, st, 0, 17)
                if i >= 1:
                    conv2_half(seq[i - 1], 1)
                conv1_chunks(vb, st, 17, 20)
                pool1_part(st, 0)
                if i >= 1:
                    emit_finishB(seq[i - 1])
                conv2_half(vb, 0)
                pool2_half(vb, 0)
                conv1_chunks(vb, st, 20, 24)
                pool1_part(st, 1)
            conv2_half(seq[-1], 1)
            emit_finishB(seq[-1])
            for g in sorted(grp):
                done = grp[g].get("tailed", 0)
                if done < grp[g]["n"]:
                    emit_tailB(g, done, grp[g]["n"])

    nc.compile()
    return nc


def _get_nc(n_blocks=NBLK, n_reps=1):
    key = ("nc", n_blocks, n_reps)
    if key not in _CACHE:
        _CACHE[key] = _build_bass(n_blocks, n_reps)
    return _CACHE[key]


def kernel(n_reps=1, **inputs):
    x = np.asarray(inputs["x"], np.float32)
    wm = _build_weight_mats(inputs["k1"], inputs["k2"], inputs["k3"],
                            inputs["W1"], inputs["b1"], inputs["W2"], inputs["b2"])
    nc = _get_nc(NBLK, n_reps)

    from concourse.bass_utils import run_bass_kernel_spmd

    in_maps = []
    for core in range(NCORES):
        xc = _relayout_x(x[core * BPC:(core + 1) * BPC].reshape(BPC, 3, 32, 32))
        m = {"x": xc}
        m.update(wm)
        in_maps.append(m)

    res = run_bass_kernel_spmd(nc, in_maps, core_ids=list(range(NCORES)))
    _CACHE["last_result"] = res
    out = np.concatenate([r["out"] for r in res.results], axis=0)
    return out.astype(np.float32)
